# revision 1
# baseline (speedup 1.0000x reference)
"""Trainium2 Bass kernel for 3-layer residual LSTM decoder (B=64,T=1024,H=768).

v2: layer-pipeline across cores 0/1/2 (lags 1/3/5 slots). Each layer core
computes its OWN input-projection (bulk) GEMM from its locally-landed input,
so the only cross-core traffic is the layer output x_l (3.1MB bf16 per
32-step window), moved by pairwise AllGathers (groups [[0,1],[2,3],..] for
hop 0->1 and [[1,2],..,[0,7]] for hop 1->2), split into half-window
transfers (h0 fired mid-slot, h1 at the next slot start).

Per step: 6 id-matmuls add precomputed pre into PSUM (start=True), then 36
bf16 recurrent matmuls accumulate h@WhhT. Gate columns are permuted
[i,f,o,g] per 1536-wide block so each block needs one sigmoid ACT (1152)
and one tanh ACT (384). Even/odd steps pack into partition halves 0:64 /
64:128 of the same 6 PSUM banks. Bulk chunks interleave between the
recurrent stream and the h-transposes to keep PE busy through the ACT/DVE
tail; the projection GEMM runs as a static burst in the slot tail.
c/h updates split across DVE (block0) and Pool (block1).
"""

import numpy as np
import ml_dtypes

import concourse.bass as bass
import concourse.tile as tile
from concourse import bacc, mybir
from concourse import bass_utils

F32 = mybir.dt.float32
BF16 = mybir.dt.bfloat16
Mult = mybir.AluOpType.mult
Add = mybir.AluOpType.add

B = 64
T_FULL = 1024
IN = 512
H = 768
G = 4 * H            # 3072
OUT = 100
NCORES = 8

HC = H // 128        # 6
KIN = IN // 128      # 4
NBLK = 2
BLKW = G // NBLK     # 1536
QW = BLKW // 4       # 384 per gate within block
WS = 32              # steps per window
HSTEP = WS // 2      # 16
ROWS_W = B * WS      # 2048
HROWS = B * HSTEP    # 1024
MT = ROWS_W // 128   # 16 Mtiles per window
LAGS = (1, 3, 5)

Sig = mybir.ActivationFunctionType.Sigmoid
Tanh = mybir.ActivationFunctionType.Tanh

GROUPS = [[0, 1, 2, 3], [4, 5, 6, 7]]


def gate_perm():
    """perm[n] = original gate column of permuted column n. Block bb holds
    [i,f,o,g] each 384 wide for h-cols [384bb, 384bb+384)."""
    n = np.arange(G)
    bb = n // BLKW
    r = n % BLKW
    q = r // QW
    m = r % QW
    qmap = np.array([0, 1, 3, 2])  # permuted slot -> original gate idx
    return (qmap[q] * H + bb * QW + m).astype(np.int64)


def build_kernel(t=T_FULL, skip_ag=False):
    nwin = t // WS
    nprog = nwin + LAGS[2]

    nc = bacc.Bacc("TRN2", target_bir_lowering=False, debug=False,
                   num_devices=NCORES)

    xT = nc.dram_tensor("xT", [KIN, 128, B * t], BF16, kind="ExternalInput")
    whhT = nc.dram_tensor("whhT", [H, G], BF16, kind="ExternalInput")
    wihT = nc.dram_tensor("wihT", [H, G], BF16, kind="ExternalInput")
    biasrep = nc.dram_tensor("biasrep", [128, G], BF16, kind="ExternalInput")
    wpT = nc.dram_tensor("wpT", [H, OUT], BF16, kind="ExternalInput")
    id64 = nc.dram_tensor("id64", [128, B], BF16, kind="ExternalInput")
    idf = nc.dram_tensor("idf", [B, B], F32, kind="ExternalInput")
    alpha = nc.dram_tensor("alpha", [128, 1], F32, kind="ExternalInput")
    hscale = nc.dram_tensor("hscale", [128, nprog], F32, kind="ExternalInput")
    out = nc.dram_tensor("out", [B * t, OUT], F32, kind="ExternalOutput")
    scratch_out = nc.dram_tensor("scratch_out", [ROWS_W, OUT], F32,
                                 kind="Internal")

    with tile.TileContext(nc) as tc:
        with (
            tc.tile_pool(name="const", bufs=1) as constp,
            tc.tile_pool(name="state", bufs=1) as statep,
            tc.tile_pool(name="pre", bufs=2) as prep,
            tc.tile_pool(name="gact", bufs=2) as gactp,
            tc.tile_pool(name="small", bufs=2) as smallp,
            tc.tile_pool(name="lhst", bufs=3) as lhstp,
            tc.tile_pool(name="bulko", bufs=2) as bulkop,
            tc.tile_pool(name="res", bufs=2) as resp,
            tc.tile_pool(name="gpsum", bufs=1, space="PSUM") as gpsump,
            tc.tile_pool(name="spsum", bufs=2, space="PSUM") as spsump,
            tc.tile_pool(name="dram", bufs=1, space="DRAM") as dramp,
        ):
            # ---------------- persistent SBUF ----------------
            whh_sb = constp.tile([128, HC, NBLK, BLKW], BF16)
            for kc in range(HC):
                nc.sync.dma_start(
                    whh_sb[:, kc],
                    whhT[128 * kc:128 * (kc + 1), :]
                    .rearrange("p (b w) -> p b w", b=NBLK))
            wih_sb = constp.tile([128, HC, G], BF16)
            for kc in range(HC):
                nc.sync.dma_start(wih_sb[:, kc, :],
                                  wihT[128 * kc:128 * (kc + 1), :])
            bias_sb = constp.tile([128, G], BF16)
            nc.sync.dma_start(bias_sb[:], biasrep[:])
            wp_sb = constp.tile([128, HC, OUT], BF16)
            for kc in range(HC):
                nc.sync.dma_start(wp_sb[:, kc, :],
                                  wpT[128 * kc:128 * (kc + 1), :])
            id_sb = constp.tile([128, B], BF16)
            nc.sync.dma_start(id_sb[:], id64[:])
            actdummy = constp.tile([1, 4], F32)
            idf_sb = constp.tile([B, B], F32)
            nc.sync.dma_start(idf_sb[:], idf[:])
            alpha_sb = constp.tile([128, 1], F32)
            nc.sync.dma_start(alpha_sb[:], alpha[:])
            hscale_sb = constp.tile([128, nprog], F32)
            nc.sync.dma_start(hscale_sb[:], hscale[:])

            c_sb = statep.tile([B, NBLK, QW], F32)
            nc.vector.memset(c_sb[:], 0.0)
            hfull = [statep.tile([B, QW], F32, name=f"hfull{bb}")
                     for bb in range(NBLK)]
            for bb in range(NBLK):
                nc.vector.memset(hfull[bb][:], 0.0)
            hTr = statep.tile([128, HC, B], BF16)
            nc.vector.memset(hTr[:], 0.0)
            xout_win = statep.tile([128, HC, ROWS_W], BF16)

            # persistent gate PSUM: even steps partitions 0:64, odd 64:128
            # (separate tiles per block so ACT of block0 doesn't wait on
            # block1's matmuls via coarse tile-level dep tracking)
            gates_ps = [gpsump.tile([128, BLKW], F32, name=f"gps{bb}")
                        for bb in range(NBLK)]

            sp_eng = bass.OrderedSet([mybir.EngineType.SP])
            pid = nc.partition_id(engines=sp_eng)
            sel = nc.snap((pid + 7) % 8, engines=sp_eng,
                          min_val=0, max_val=7)

            # ---------------- DRAM ring ----------------
            # staging[par, half, slot, kc, krow, col]:
            #   slots 0:4 = quad AllGather out (rank order), 7 = local x copy
            staging = dramp.tile([2, 2, 8, HC, 128, HROWS], BF16,
                                 name="staging")
            agx = [dramp.tile([2, HC, 128, HROWS], BF16, name=f"agx{h}")
                   for h in range(2)]
            pre_dram = dramp.tile([2, ROWS_W, G], BF16, name="pre_dram")

            # one-time zeroing (junk must be finite: 0*NaN=NaN would defeat
            # the hscale state reset)
            ztile = prep.tile([128, G], BF16, tag="pre", name="ztile")
            nc.vector.memset(ztile[:], 0.0)
            for buf in (staging, pre_dram):
                v = buf[:].flatten().rearrange("(p n) -> p n", p=128)
                ncols = v.shape[1]
                off = 0
                while off < ncols:
                    w = min(G, ncols - off)
                    nc.sync.dma_start(v[:, off:off + w], ztile[:, 0:w])
                    off += w

            lh_bulk = None   # current body's bulk lhs tile
            ob_bulk = None   # current body's batched bulk output tile
            resh_cur = None  # current half's residual/x window [128,HC,HROWS]

            def bulk_chunk(n):
                sp = spsump.tile([128, 512], F32, tag="sp")
                for k in range(HC):
                    nc.tensor.matmul(
                        sp[:], lh_bulk[:, k, :],
                        wih_sb[:, k, 512 * n:512 * (n + 1)],
                        start=(k == 0), stop=(k == HC - 1))
                nc.vector.scalar_tensor_tensor(
                    ob_bulk[:, 512 * n:512 * (n + 1)], sp[:], 1.0,
                    bias_sb[:, 512 * n:512 * (n + 1)], Mult, Add)

            def emit_step(iv, j, hf, par, pre_t):
                """One LSTM step at loop index iv, substep j, half hf."""
                tvh = iv * 2 + j            # step index within half
                tv = tvh + HSTEP * hf       # step index within window
                prow = 64 * j               # psum partition offset

                ps = [gates_ps[bb][prow:prow + 64] for bb in range(NBLK)]

                # pre into PSUM (opens the accumulation groups); identity
                # rows match the pre rows (step u: 0:64, step v: 64:128)
                idl = id_sb[prow:prow + 64, :]
                prer = pre_t[prow:prow + 64].rearrange(
                    "r (b w) -> r b w", b=NBLK)
                for bb in range(NBLK):
                    for ch in range(3):
                        nc.tensor.matmul(
                            ps[bb][:, 512 * ch:512 * (ch + 1)], idl,
                            prer[:, bb, 512 * ch:512 * (ch + 1)],
                            start=True, stop=False)
                # recurrent accumulation, block-major so block0 closes early
                for bb in range(NBLK):
                    for k in range(HC):
                        lh = hTr[:, k, :]
                        for ch in range(3):
                            nc.tensor.matmul(
                                ps[bb][:, 512 * ch:512 * (ch + 1)], lh,
                                whh_sb[:, k, bb, 512 * ch:512 * (ch + 1)],
                                start=False, stop=(k == HC - 1))

                # activations and c/h updates (all elementwise on DVE)
                gct = [gactp.tile([B, BLKW], F32, tag=f"gact{bb}",
                                  name=f"gct{bb}")
                       for bb in range(NBLK)]
                E = nc.vector
                for bb in range(NBLK):
                    g = gct[bb]
                    nc.scalar.activation(g[:, 0:3 * QW],
                                         ps[bb][:, 0:3 * QW], Sig)
                    nc.scalar.activation(g[:, 3 * QW:BLKW],
                                         ps[bb][:, 3 * QW:BLKW], Tanh)
                    ig = smallp.tile([B, QW], F32, tag=f"ig{bb}")
                    E.tensor_mul(ig[:], g[:, 0:QW], g[:, 3 * QW:BLKW])
                    E.tensor_mul(c_sb[:, bb, :], g[:, QW:2 * QW],
                                 c_sb[:, bb, :])
                    E.tensor_add(c_sb[:, bb, :], c_sb[:, bb, :], ig[:])
                    tcb = smallp.tile([B, QW], F32, tag=f"tc{bb}")
                    nc.scalar.activation(tcb[:], c_sb[:, bb, :], Tanh)
                    E.tensor_mul(hfull[bb][:], g[:, 2 * QW:3 * QW], tcb[:])

                def t_block(bb):
                    # transpose h block bb into hTr, then xout k-rows
                    sp = spsump.tile([128, 512], F32, tag="sp")
                    for q in range(3):
                        nc.tensor.transpose(
                            sp[:, 64 * q:64 * (q + 1)],
                            hfull[bb][:, 128 * q:128 * (q + 1)], idf_sb[:])
                    nc.vector.tensor_copy(
                        hTr[:, 3 * bb:3 * bb + 3, :]
                        .rearrange("p a b -> p (a b)"), sp[:, 0:192])
                    nc.vector.scalar_tensor_tensor(
                        xout_win[:, 3 * bb:3 * bb + 3, bass.ts(tv, B)],
                        resh_cur[:, 3 * bb:3 * bb + 3, bass.ts(tvh, B)],
                        alpha_sb[:, 0:1],
                        hTr[:, 3 * bb:3 * bb + 3, :],
                        Mult, Add)

                # PE fillers between the recurrent stream and transposes
                bulk_chunk(3 * j + 0)
                t_block(0)
                bulk_chunk(3 * j + 1)
                bulk_chunk(3 * j + 2)
                t_block(1)

            # ---------------- program slots ----------------
            for p in range(nprog):
                par = p % 2

                if p >= 1 and not skip_ag:
                    nc.gpsimd.collective_compute(
                        "AllGather", mybir.AluOpType.bypass,
                        replica_groups=GROUPS,
                        ins=[agx[1][1 - par]],
                        outs=[staging[par, 1, 0:4]])

                # scale state at window start (zero at my first real window)
                nc.vector.tensor_scalar_mul(hTr[:], hTr[:],
                                            hscale_sb[:, p:p + 1])
                nc.vector.tensor_scalar_mul(c_sb[:], c_sb[:],
                                            hscale_sb[0:B, p:p + 1])
                # dummy activations so the table-load analysis sees both
                # functions live on every path into the step loops
                nc.scalar.activation(actdummy[:], actdummy[:], Sig)
                nc.scalar.activation(actdummy[:], actdummy[:], Tanh)

                # local x window into staging slot 7 (k-tiles 4:6 stay zero)
                c0 = min(p, nwin - 1) * ROWS_W
                nc.gpsimd.dma_start(staging[1 - par, 0, 7, 0:KIN],
                                    xT[:, :, c0:c0 + HROWS])
                nc.gpsimd.dma_start(staging[par, 1, 7, 0:KIN],
                                    xT[:, :, c0 + HROWS:c0 + ROWS_W])

                def run_half(hf):
                    nonlocal resh_cur
                    spar = par if hf == 0 else 1 - par
                    resh = resp.tile([128, HC, HROWS], BF16, tag="resh")
                    nc.sync.dma_start(
                        resh[:],
                        staging[spar, hf, bass.ds(sel, 1), :, :, :]
                        .transpose([2, 1, 0, 3]).squeeze(2))
                    resh_cur = resh

                    def body(iv):
                        nonlocal lh_bulk, ob_bulk
                        mtile = iv if hf == 0 else iv + 8
                        lh_bulk = lhstp.tile([128, HC, 128], BF16, tag="lh")
                        bpar = (1 - par) if hf == 0 else par
                        nc.sync.dma_start(
                            lh_bulk[:],
                            staging[bpar, hf, bass.ds(sel, 1), :, :,
                                    bass.ts(iv, 128)]
                            .transpose([2, 1, 0, 3]).squeeze(2))
                        pre_t = prep.tile([128, G], BF16, tag="pre")
                        nc.sync.dma_start(
                            pre_t[:],
                            pre_dram[1 - par,
                                     bass.ds((iv * 2 + 16 * hf) * B, 128), :])
                        ob_bulk = bulkop.tile([128, G], BF16, tag="bob")
                        emit_step(iv, 0, hf, par, pre_t)
                        emit_step(iv, 1, hf, par, pre_t)
                        nc.gpsimd.dma_start(
                            pre_dram[par, bass.ds(mtile * 128, 128), :],
                            ob_bulk[:])
                    tc.For_i_unrolled(0, 8, 1, body, 1)

                run_half(0)

                # mid-slot: ship xout h0, fire AG h0
                for kc in range(HC):
                    nc.gpsimd.dma_start(agx[0][par, kc],
                                        xout_win[:, kc, 0:HROWS])
                if p >= 1 and not skip_ag:
                    nc.gpsimd.collective_compute(
                        "AllGather", mybir.AluOpType.bypass,
                        replica_groups=GROUPS,
                        ins=[agx[0][par]],
                        outs=[staging[par, 0, 0:4]])

                run_half(1)

                # tail: ship xout h1; projection burst (static Mtiles)
                for kc in range(HC):
                    nc.gpsimd.dma_start(agx[1][par, kc],
                                        xout_win[:, kc, HROWS:ROWS_W])

                w2 = p - LAGS[2]
                in_range = 0 <= w2 < nwin
                for m in range(MT):
                    sp = spsump.tile([128, 512], F32, tag="sp")
                    for k in range(HC):
                        nc.tensor.matmul(
                            sp[:, 0:OUT],
                            xout_win[:, k, 128 * m:128 * (m + 1)],
                            wp_sb[:, k, :], start=(k == 0),
                            stop=(k == HC - 1))
                    po = bulkop.tile([128, OUT], F32, tag="po")
                    nc.vector.tensor_copy(po[:], sp[:, 0:OUT])
                    if in_range:
                        nc.gpsimd.dma_start(
                            out[w2 * ROWS_W + 128 * m:
                                w2 * ROWS_W + 128 * (m + 1), :], po[:])
                    else:
                        nc.gpsimd.dma_start(
                            scratch_out[128 * m:128 * (m + 1), :], po[:])

    nc.compile()
    return nc


# ---------------- host-side glue ----------------
def prep_inputs(x, Wih1, Whh1, b1, Wih2, Whh2, b2, Wih3, Whh3, b3, Wp,
                t=T_FULL):
    nwin = t // WS
    nprog = nwin + LAGS[2]
    perm = gate_perm()
    bf = ml_dtypes.bfloat16

    x = np.asarray(x, np.float32)[:, :t]
    xTp = np.ascontiguousarray(
        np.transpose(x, (2, 1, 0)).reshape(KIN, 128, t * B)).astype(bf)
    xz = np.zeros_like(xTp)

    def permT(w):
        return np.ascontiguousarray(np.asarray(w).T[:, perm]).astype(bf)

    wih1p = np.zeros((H, G), np.float32)
    wih1p[:IN] = np.asarray(Wih1).T
    wih1p = np.ascontiguousarray(wih1p[:, perm]).astype(bf)
    wihs = {0: wih1p, 1: permT(Wih2), 2: permT(Wih3)}
    whhs = {0: permT(Whh1), 1: permT(Whh2), 2: permT(Whh3)}
    biases = {0: np.asarray(b1), 1: np.asarray(b2), 2: np.asarray(b3)}
    zeroW = np.zeros((H, G), bf)
    zeroB = np.zeros((128, G), bf)

    wpT = np.ascontiguousarray(np.asarray(Wp).T).astype(bf)

    in_maps = []
    for c in range(NCORES):
        al = np.full((128, 1), 1.0 if c in (1, 2) else 0.0, np.float32)
        hs = np.ones((128, nprog), np.float32)
        if c <= 2:
            hs[:, LAGS[c]] = 0.0
        else:
            hs[:] = 0.0
        brep = zeroB
        if c <= 2:
            brep = np.ascontiguousarray(
                np.tile(biases[c][perm][None, :], (128, 1))).astype(bf)
        in_maps.append({
            "xT": xTp if c == 0 else xz,
            "whhT": whhs.get(c, zeroW),
            "wihT": wihs.get(c, zeroW),
            "biasrep": brep,
            "wpT": wpT,
            "id64": np.vstack([np.eye(B), np.eye(B)]).astype(bf),
            "idf": np.eye(B, dtype=np.float32),
            "alpha": al,
            "hscale": hs,
        })
    return in_maps


_NC_CACHE = {}


def kernel(**inputs):
    if "nc" not in _NC_CACHE:
        _NC_CACHE["nc"] = build_kernel()
    nc = _NC_CACHE["nc"]
    in_maps = prep_inputs(**inputs)
    res = bass_utils.run_bass_kernel_spmd(nc, in_maps,
                                          core_ids=list(range(NCORES)))
    o = res.results[2]["out"]
    return np.ascontiguousarray(
        o.reshape(T_FULL, B, OUT).transpose(1, 0, 2)).astype(np.float32)

